# revision 5
# baseline (speedup 1.0000x reference)
"""Trainium2 Bass kernel for 3-layer CuGraphSAGE on a fanout-8 sampled tree.

The sampled graph is a forest of B=4096 independent trees (children of
parent p are rows [4096+8p, 4096+8p+8)). Shard by seed block: core c gets
512 seeds plus their full 3-hop subtrees (contiguous row blocks, exactly
1/8 of all rows, zero halo, no collectives).

Precision: leaf-hop features (87.5% of bytes) stream as fp8 e3m4, inner
hops as bf16, weights bf16, PSUM accumulation f32. Measured end-to-end
rel err ~4e-3 vs the f32 reference.

Layout: channel-major [128ch, rows] so the matmul contraction dim is the
partition dim. Leaf columns are host-reordered PLANE-major per outer tile
(all sibling-0 cols, then sibling-1, ...) so the mean-aggregation is 8
accumulating matmuls with fully CONTIGUOUS 512-col rhs slabs -- strided
rhs runs 2-4x slower on the PE, contiguous hits 216ns/512col. The 1/8 is
folded into the aggregation weight. Inner aggregations (natural order,
sibling-adjacent) use the DVE reduce (4.4us/tile, dtype-independent); a
few leaf sub-tiles are offloaded to DVE via packed plane-adds to balance
engines. Only x is streamed from HBM (~43 MB/core).
"""

import os
import numpy as np

# ---------------------------------------------------------------- constants
N_CORES = 8
C = 128                       # channels
B = 4096                      # seeds
S = B // N_CORES              # 512 seeds per core
BLK = [512, 4096, 32768, 262144]          # per-core rows per hop
OFF = [0, 4096, 36864, 299008]            # global start row of each hop block
NIN = BLK[0] + BLK[1] + BLK[2]            # 37376 inner rows (hop 0-2)
NLEAF = BLK[3]                            # 262144 leaf rows
NPAR1 = BLK[0] + BLK[1]                   # 4608 layer-1 parents
PT = 512                                  # parents per PSUM tile
LB = 8 * BLK[1]                           # 32768 leaf rows per outer tile
N_FULL = 2396160
E_FULL = 2392064
OUT_ROWS = 36864

TRACE = os.environ.get("GNN_TRACE", "0") == "1"
# number of leaf sub-tiles (of 64) aggregated on DVE instead of PE
NDVE = int(os.environ.get("GNN_NDVE", "8"))
INNER8 = os.environ.get("GNN_INNER8", "0") == "1"   # inner hops fp8e3 too
LAST_RESULT = None

_BASS_CACHE = {}


def _leaf_dve_flags(ndve):
    # spread ndve True flags evenly over the 64 leaf sub-tiles
    return [(m * ndve) // 64 != ((m + 1) * ndve) // 64 for m in range(64)]


def _build_bass(ndve, inner8):
    import concourse.mybir as mybir
    from concourse import bacc
    from concourse.tile import TileContext

    bf16 = mybir.dt.bfloat16
    f8e3 = mybir.dt.float8e3
    f32 = mybir.dt.float32
    Relu = mybir.ActivationFunctionType.Relu
    AxX = mybir.AxisListType.X
    Add = mybir.AluOpType.add

    dve_flag = _leaf_dve_flags(ndve)
    dt_in = f8e3 if inner8 else bf16

    nc = bacc.Bacc()
    xiT = nc.dram_tensor("xiT", [C, NIN], dt_in, kind="ExternalInput")
    xlT = nc.dram_tensor("xlT", [C, NLEAF], f8e3, kind="ExternalInput")
    wconsts = nc.dram_tensor("wconsts", [C, 6 * C], bf16, kind="ExternalInput")
    bconsts = nc.dram_tensor("bconsts", [C, 3], f32, kind="ExternalInput")
    out = nc.dram_tensor("out", [C, NPAR1], f32, kind="ExternalOutput")
    WIDX = {k: i for i, k in
            enumerate(("w1a", "w1b", "w2a", "w2b", "w3a", "w3b"))}

    with TileContext(nc) as tc:
        with tc.tile_pool(name="const", bufs=1) as constp, \
             tc.tile_pool(name="keep", bufs=1) as keepp, \
             tc.tile_pool(name="cbuf", bufs=2) as cpool, \
             tc.tile_pool(name="dbuf", bufs=4) as dpool, \
             tc.tile_pool(name="hbuf", bufs=2) as hpool, \
             tc.tile_pool(name="obuf", bufs=2) as opool, \
             tc.tile_pool(name="aggbuf", bufs=3) as aggp, \
             tc.tile_pool(name="addbuf", bufs=8) as addp, \
             tc.tile_pool(name="ps", bufs=8, space="PSUM") as pp:

            wtile = constp.tile([C, 6 * C], bf16, name="wtile")
            nc.sync.dma_start(wtile[:, :], wconsts[:, :])
            btile = constp.tile([C, 3], f32, name="btile")
            nc.sync.dma_start(btile[:, :], bconsts[:, :])
            w = {k: wtile[:, C * i: C * (i + 1)] for k, i in WIDX.items()}
            bt = {f"b{i+1}": btile[:, i: i + 1] for i in range(3)}

            xA01 = keepp.tile([C, NPAR1], dt_in, tag="xA01")
            nc.sync.dma_start(xA01[:, :], xiT[:, 0:NPAR1])
            h1self = keepp.tile([C, NPAR1], bf16, tag="h1self")
            h2sb = keepp.tile([C, NPAR1], bf16, tag="h2sb")
            # incrementally-built aggregations for the drain tiles
            agg1z = keepp.tile([C, PT], bf16, tag="agg1z")   # L1 tile 0
            agg18 = keepp.tile([C, PT], bf16, tag="agg18")   # L1 tile 8
            agg2 = keepp.tile([C, PT], bf16, tag="agg2")     # L2

            def dve_reduce(children_ap, tag, name):
                # DVE group-reduce over sibling-adjacent natural order
                aggt = aggp.tile([C, PT], bf16, tag=tag, name=name)
                with nc.allow_low_precision(reason="8-term sibling sum"):
                    nc.vector.reduce_sum(
                        aggt[:, :],
                        children_ap.rearrange("c (p e) -> c p e", e=8),
                        axis=AxX)
                return aggt

            def chunk_reduce(dst_ap, children_ap):
                # 64-parent partial group-reduce into a persistent agg tile
                with nc.allow_low_precision(reason="8-term sibling sum"):
                    nc.vector.reduce_sum(
                        dst_ap,
                        children_ap.rearrange("c (p e) -> c p e", e=8),
                        axis=AxX)

            def agg_mms(psum, wa, wb, aggt, self_ap):
                nc.tensor.matmul(psum, w[wa], aggt[:, :],
                                 start=True, stop=False)
                nc.tensor.matmul(psum, w[wb], self_ap,
                                 start=False, stop=True)

            def dve_sage(psum, wa, wb, children_ap, self_ap):
                aggt = dve_reduce(children_ap, "agg", "aggt")
                agg_mms(psum, wa, wb, aggt, self_ap)

            HW2 = 4 * PT                       # 2048 parents per half

            def leaf_slab(Dx, v, e):
                return Dx[:, HW2 * e + PT * v: HW2 * e + PT * (v + 1)]

            def leaf_pe_sage(psum, Dx, v, self_ap):
                # 8 accumulating mms over contiguous plane slabs
                for e in range(8):
                    nc.tensor.matmul(psum, w["w1a"], leaf_slab(Dx, v, e),
                                     start=(e == 0), stop=False)
                nc.tensor.matmul(psum, w["w1b"], self_ap,
                                 start=False, stop=True)

            def leaf_dve_sage(psum, Dx, v, self_ap):
                # packed plane-adds: 4x (fp8+fp8->bf16), then 2+1 bf16
                def sl(e):
                    return leaf_slab(Dx, v, e)
                with nc.allow_low_precision(reason="8-term sibling sum"):
                    t4 = [addp.tile([C, PT], bf16, tag="add", name=f"t4_{j}")
                          for j in range(4)]
                    for j in range(4):
                        nc.vector.tensor_tensor(
                            t4[j][:, :], sl(2 * j), sl(2 * j + 1), op=Add)
                    s0 = addp.tile([C, PT], bf16, tag="add", name="s0")
                    nc.vector.tensor_tensor(s0[:, :], t4[0][:, :],
                                            t4[1][:, :], op=Add)
                    s1 = addp.tile([C, PT], bf16, tag="add", name="s1")
                    nc.vector.tensor_tensor(s1[:, :], t4[2][:, :],
                                            t4[3][:, :], op=Add)
                    aggt = aggp.tile([C, PT], bf16, tag="agg", name="aggd")
                    nc.vector.tensor_tensor(aggt[:, :], s0[:, :],
                                            s1[:, :], op=Add)
                nc.tensor.matmul(psum, w["w1a"], aggt[:, :],
                                 start=True, stop=False)
                nc.tensor.matmul(psum, w["w1b"], self_ap,
                                 start=False, stop=True)

            def w3b_tile(t):
                # h2 rows [512t, 512(t+1)) have no in-edges: self term only
                psn = pp.tile([C, PT], f32, tag="ps", name=f"psn{t}")
                nc.tensor.matmul(psn, w["w3b"],
                                 h2sb[:, PT * t: PT * (t + 1)],
                                 start=True, stop=True)
                on = opool.tile([C, PT], f32, tag="o", name=f"on{t}")
                nc.scalar.activation(on[:, :], psn, Relu, bias=bt["b3"])
                nc.sync.dma_start(out[:, PT * t: PT * (t + 1)], on[:, :])

            # layer-0 tile 0 (seeds) early: only needs xA01; fills DVE
            # while the first Ct/D DMAs stream.
            ps0z = pp.tile([C, PT], f32, tag="ps", name="ps0z")
            dve_sage(ps0z, "w1a", "w1b", xA01[:, S:NPAR1], xA01[:, 0:S])
            nc.scalar.activation(h1self[:, 0:S], ps0z, Relu, bias=bt["b1"])

            # Software-pipelined main loop: tile t's DVE-dependent matmuls
            # (layer-1 of t-1, layer-0-inner of t) are emitted AFTER tile
            # t's leaf matmul burst so the PE never waits on a reduce.
            h1tmp_prev = None
            for t in range(1, 9):
                Ct = cpool.tile([C, 8 * PT], dt_in, tag="C")
                nc.sync.dma_start(
                    Ct[:, :], xiT[:, S + 8 * PT * t: S + 8 * PT * (t + 1)])
                Dh = []
                for h in range(2):
                    Dx = dpool.tile([C, LB // 2], f8e3, tag="D",
                                    name=f"D{t}_{h}")
                    nc.sync.dma_start(
                        Dx[:, :], xlT[:, LB * (t - 1) + (LB // 2) * h:
                                      LB * (t - 1) + (LB // 2) * (h + 1)])
                    Dh.append(Dx)

                # DVE queue: L1(t-1) reduce (input ready), L0(t) reduce
                if h1tmp_prev is not None:
                    agg1p = dve_reduce(h1tmp_prev[:, :], "agg1", f"a1_{t}")
                agg0 = dve_reduce(Ct[:, :], "agg0", f"a0_{t}")

                # PE queue: ready-first. w3b(t-2), then the leaf burst.
                if t >= 3:
                    w3b_tile(t - 2)

                h1tmp = hpool.tile([C, 8 * PT], bf16, tag="h1tmp")
                for u in range(8):
                    psu = pp.tile([C, PT], f32, tag="ps", name=f"psu{t}_{u}")
                    if dve_flag[8 * (t - 1) + u]:
                        leaf_dve_sage(psu, Dh[u // 4], u % 4,
                                      Ct[:, PT * u: PT * (u + 1)])
                    else:
                        leaf_pe_sage(psu, Dh[u // 4], u % 4,
                                     Ct[:, PT * u: PT * (u + 1)])
                    nc.scalar.activation(h1tmp[:, PT * u: PT * (u + 1)], psu,
                                         Relu, bias=bt["b1"])
                    if t == 8:
                        chunk_reduce(agg18[:, 64 * u: 64 * (u + 1)],
                                     h1tmp[:, PT * u: PT * (u + 1)])

                # layer-0 tile for parents [512t, 512(t+1)) (hop-1 nodes)
                ps0 = pp.tile([C, PT], f32, tag="ps", name=f"ps0_{t}")
                agg_mms(ps0, "w1a", "w1b", agg0,
                        xA01[:, PT * t: PT * (t + 1)])
                nc.scalar.activation(h1self[:, PT * t: PT * (t + 1)], ps0,
                                     Relu, bias=bt["b1"])
                chunk_reduce(agg1z[:, 64 * (t - 1): 64 * t],
                             h1self[:, PT * t: PT * (t + 1)])

                # layer-1 tile for parents [512(t-1), 512t) -> h2
                if h1tmp_prev is not None:
                    ps1 = pp.tile([C, PT], f32, tag="ps", name=f"ps1_{t}")
                    agg_mms(ps1, "w2a", "w2b", agg1p,
                            h1self[:, PT * (t - 1): PT * t])
                    nc.scalar.activation(h2sb[:, PT * (t - 1): PT * t], ps1,
                                         Relu, bias=bt["b2"])
                    chunk_reduce(agg2[:, 64 * (t - 2): 64 * (t - 1)],
                                 h2sb[:, PT * (t - 1): PT * t])
                h1tmp_prev = h1tmp

            # drain: all aggregations were chunk-built in the loop
            # layer-1 tile 0 (uses agg1z, complete after ps0(8) ACT)
            ps1z = pp.tile([C, PT], f32, tag="ps", name="ps1z")
            agg_mms(ps1z, "w2a", "w2b", agg1z, h1self[:, 0:S])
            nc.scalar.activation(h2sb[:, 0:S], ps1z, Relu, bias=bt["b2"])
            w3b_tile(7)

            # layer-1 tile 8 (agg18 chunk-built during tile 8)
            ps1 = pp.tile([C, PT], f32, tag="ps", name="ps1_9")
            agg_mms(ps1, "w2a", "w2b", agg18, h1self[:, 8 * PT: 9 * PT])
            nc.scalar.activation(h2sb[:, 8 * PT: 9 * PT], ps1,
                                 Relu, bias=bt["b2"])
            chunk_reduce(agg2[:, 64 * 7: 64 * 8],
                         h2sb[:, 8 * PT: 9 * PT])
            w3b_tile(8)

            # layer 2: parents [0, 512) aggregate h2[512:4608)
            ps2 = pp.tile([C, PT], f32, tag="ps", name="ps2")
            agg_mms(ps2, "w3a", "w3b", agg2, h2sb[:, 0:S])
            o0 = opool.tile([C, PT], f32, tag="o", name="o0")
            nc.scalar.activation(o0[:, :], ps2, Relu, bias=bt["b3"])
            nc.sync.dma_start(out[:, 0:S], o0[:, :])

    nc.compile()
    return nc


def _get_bass(ndve, inner8):
    key = (ndve, inner8)
    if key not in _BASS_CACHE:
        _BASS_CACHE[key] = _build_bass(ndve, inner8)
    return _BASS_CACHE[key]


def _edge_is_tree(edge):
    if edge.shape != (2, E_FULL):
        return False
    ar = np.arange(E_FULL, dtype=np.int64)
    return (np.array_equal(edge[0], (B + ar).astype(np.int32))
            and np.array_equal(edge[1], (ar // 8).astype(np.int32)))


def _fallback(x, edge, W1, b1, W2, b2, W3, b3):
    # General (structure-agnostic) CPU implementation; only used if the
    # inputs are not the fanout-8 tree this kernel is specialized for.
    sizes = [(N_FULL, E_FULL), (299008, 294912), (36864, 32768)]
    params = [(W1, b1), (W2, b2), (W3, b3)]
    x = x.astype(np.float32)
    for (n, e), (Wl, bl) in zip(sizes, params):
        src = edge[0, :e].astype(np.int64)
        dst = edge[1, :e].astype(np.int64)
        x = x[:n]
        agg = np.zeros((n, x.shape[1]), np.float32)
        np.add.at(agg, dst, x[src])
        deg = np.bincount(dst, minlength=n).astype(np.float32)
        agg /= np.maximum(deg, 1.0)[:, None]
        x = np.maximum(np.concatenate([agg, x], axis=1) @ Wl.T + bl, 0.0)
    return x


def kernel(**inputs):
    global LAST_RESULT
    x = np.asarray(inputs["x"])
    edge = np.asarray(inputs["edge"])
    W = [np.asarray(inputs[k], dtype=np.float32) for k in ("W1", "W2", "W3")]
    bias = [np.asarray(inputs[k], dtype=np.float32) for k in ("b1", "b2", "b3")]

    if x.shape != (N_FULL, C) or not _edge_is_tree(edge):
        return _fallback(x, edge, W[0], bias[0], W[1], bias[1], W[2], bias[2])

    import ml_dtypes
    from concourse.bass_utils import run_bass_kernel_spmd

    bf16 = ml_dtypes.bfloat16
    f8e3 = ml_dtypes.float8_e3m4
    x = np.ascontiguousarray(x, dtype=np.float32)

    wblocks = []
    for li in range(3):
        wblocks.append((W[li][:, :C] / 8.0).T)     # agg part, mean folded in
        wblocks.append(W[li][:, C:].T)             # self part
    wconsts = np.concatenate(wblocks, axis=1).astype(bf16)
    wconsts = np.ascontiguousarray(wconsts)
    bconsts = np.ascontiguousarray(np.stack(bias, axis=1))      # [128, 3] f32

    in_maps = []
    for c in range(N_CORES):
        xi = np.concatenate(
            [x[OFF[h] + BLK[h] * c: OFF[h] + BLK[h] * (c + 1)]
             for h in range(3)], axis=0)
        xiTc = np.ascontiguousarray(xi.T).astype(f8e3 if INNER8 else bf16, copy=False)
        xl = x[OFF[3] + BLK[3] * c: OFF[3] + BLK[3] * (c + 1)]
        # per outer tile, split parents into 2 halves; within each half,
        # plane-major: [8 tiles, 2 halves, 8 sib, 2048 parents, 128ch]
        xl = xl.reshape(8, 2, 2048, 8, C).transpose(0, 1, 3, 2, 4)
        xl = xl.reshape(-1, C)
        xlTc = np.ascontiguousarray(xl.T).astype(f8e3, copy=False)
        in_maps.append({"xiT": xiTc, "xlT": xlTc,
                        "wconsts": wconsts, "bconsts": bconsts})

    nc = _get_bass(NDVE, INNER8)
    res = run_bass_kernel_spmd(nc, in_maps, list(range(N_CORES)), trace=TRACE)
    LAST_RESULT = res

    out = np.empty((OUT_ROWS, C), np.float32)
    for c in range(N_CORES):
        oc = np.asarray(res.results[c]["out"])
        out[S * c: S * (c + 1)] = oc[:, :S].T
        out[B + 8 * S * c: B + 8 * S * (c + 1)] = oc[:, S:].T
    return out


# revision 6
# speedup vs baseline: 1.1108x; 1.1108x over previous
"""Trainium2 Bass kernel for 3-layer CuGraphSAGE on a fanout-8 sampled tree.

The sampled graph is a forest of B=4096 independent trees (children of
parent p are rows [4096+8p, 4096+8p+8)). Shard by seed block: core c gets
512 seeds plus their full 3-hop subtrees (contiguous row blocks, exactly
1/8 of all rows, zero halo, no collectives).

Precision: leaf-hop features (87.5% of bytes) stream as fp8 e3m4, inner
hops as bf16, weights bf16, PSUM accumulation f32. Measured end-to-end
rel err ~4e-3 vs the f32 reference.

Layout: channel-major [128ch, rows] so the matmul contraction dim is the
partition dim. Leaf columns are host-reordered PLANE-major per outer tile
(all sibling-0 cols, then sibling-1, ...) so the mean-aggregation is 8
accumulating matmuls with fully CONTIGUOUS 512-col rhs slabs -- strided
rhs runs 2-4x slower on the PE, contiguous hits 216ns/512col. The 1/8 is
folded into the aggregation weight. Inner aggregations (natural order,
sibling-adjacent) use the DVE reduce (4.4us/tile, dtype-independent); a
few leaf sub-tiles are offloaded to DVE via packed plane-adds to balance
engines. Only x is streamed from HBM (~43 MB/core).
"""

import os
import numpy as np

# ---------------------------------------------------------------- constants
N_CORES = 8
C = 128                       # channels
B = 4096                      # seeds
S = B // N_CORES              # 512 seeds per core
BLK = [512, 4096, 32768, 262144]          # per-core rows per hop
OFF = [0, 4096, 36864, 299008]            # global start row of each hop block
NIN = BLK[0] + BLK[1] + BLK[2]            # 37376 inner rows (hop 0-2)
NLEAF = BLK[3]                            # 262144 leaf rows
NPAR1 = BLK[0] + BLK[1]                   # 4608 layer-1 parents
PT = 512                                  # parents per PSUM tile
LB = 8 * BLK[1]                           # 32768 leaf rows per outer tile
N_FULL = 2396160
E_FULL = 2392064
OUT_ROWS = 36864

TRACE = os.environ.get("GNN_TRACE", "0") == "1"
# number of leaf sub-tiles (of 64) aggregated on DVE instead of PE
NDVE = int(os.environ.get("GNN_NDVE", "10"))
INNER8 = os.environ.get("GNN_INNER8", "0") == "1"   # inner hops fp8e3 too
LAST_RESULT = None

_BASS_CACHE = {}


def _leaf_dve_flags(ndve):
    # spread ndve True flags evenly over the 64 leaf sub-tiles
    return [(m * ndve) // 64 != ((m + 1) * ndve) // 64 for m in range(64)]


def _build_bass(ndve, inner8):
    import concourse.mybir as mybir
    from concourse import bacc
    from concourse.tile import TileContext

    bf16 = mybir.dt.bfloat16
    f8e3 = mybir.dt.float8e3
    f32 = mybir.dt.float32
    Relu = mybir.ActivationFunctionType.Relu
    AxX = mybir.AxisListType.X
    Add = mybir.AluOpType.add

    dve_flag = _leaf_dve_flags(ndve)
    dt_in = f8e3 if inner8 else bf16

    nc = bacc.Bacc()
    xiT = nc.dram_tensor("xiT", [C, NIN], dt_in, kind="ExternalInput")
    xlT = nc.dram_tensor("xlT", [C, NLEAF], f8e3, kind="ExternalInput")
    wconsts = nc.dram_tensor("wconsts", [C, 6 * C], bf16, kind="ExternalInput")
    bconsts = nc.dram_tensor("bconsts", [C, 3], f32, kind="ExternalInput")
    out = nc.dram_tensor("out", [C, NPAR1], f32, kind="ExternalOutput")
    WIDX = {k: i for i, k in
            enumerate(("w1a", "w1b", "w2a", "w2b", "w3a", "w3b"))}

    with TileContext(nc) as tc:
        with tc.tile_pool(name="const", bufs=1) as constp, \
             tc.tile_pool(name="keep", bufs=1) as keepp, \
             tc.tile_pool(name="cbuf", bufs=2) as cpool, \
             tc.tile_pool(name="dbuf", bufs=4) as dpool, \
             tc.tile_pool(name="hbuf", bufs=2) as hpool, \
             tc.tile_pool(name="obuf", bufs=2) as opool, \
             tc.tile_pool(name="aggbuf", bufs=3) as aggp, \
             tc.tile_pool(name="addbuf", bufs=8) as addp, \
             tc.tile_pool(name="ps", bufs=8, space="PSUM") as pp:

            wtile = constp.tile([C, 6 * C], bf16, name="wtile")
            nc.sync.dma_start(wtile[:, :], wconsts[:, :])
            btile = constp.tile([C, 3], f32, name="btile")
            nc.sync.dma_start(btile[:, :], bconsts[:, :])
            w = {k: wtile[:, C * i: C * (i + 1)] for k, i in WIDX.items()}
            bt = {f"b{i+1}": btile[:, i: i + 1] for i in range(3)}

            xA01 = keepp.tile([C, NPAR1], dt_in, tag="xA01")
            nc.sync.dma_start(xA01[:, :], xiT[:, 0:NPAR1])
            h1self = keepp.tile([C, NPAR1], bf16, tag="h1self")
            h2sb = keepp.tile([C, NPAR1], bf16, tag="h2sb")

            def dve_reduce(children_ap, tag, name):
                # DVE group-reduce over sibling-adjacent natural order
                aggt = aggp.tile([C, PT], bf16, tag=tag, name=name)
                with nc.allow_low_precision(reason="8-term sibling sum"):
                    nc.vector.reduce_sum(
                        aggt[:, :],
                        children_ap.rearrange("c (p e) -> c p e", e=8),
                        axis=AxX)
                return aggt


            def agg_mms(psum, wa, wb, aggt, self_ap):
                nc.tensor.matmul(psum, w[wa], aggt[:, :],
                                 start=True, stop=False)
                nc.tensor.matmul(psum, w[wb], self_ap,
                                 start=False, stop=True)

            def dve_sage(psum, wa, wb, children_ap, self_ap):
                aggt = dve_reduce(children_ap, "agg", "aggt")
                agg_mms(psum, wa, wb, aggt, self_ap)

            HW2 = 4 * PT                       # 2048 parents per half

            def leaf_slab(Dx, v, e):
                return Dx[:, HW2 * e + PT * v: HW2 * e + PT * (v + 1)]

            def leaf_pe_sage(psum, Dx, v, self_ap):
                # 8 accumulating mms over contiguous plane slabs
                for e in range(8):
                    nc.tensor.matmul(psum, w["w1a"], leaf_slab(Dx, v, e),
                                     start=(e == 0), stop=False)
                nc.tensor.matmul(psum, w["w1b"], self_ap,
                                 start=False, stop=True)

            def leaf_dve_sage(psum, Dx, v, self_ap):
                # packed plane-adds: 4x (fp8+fp8->bf16), then 2+1 bf16
                def sl(e):
                    return leaf_slab(Dx, v, e)
                with nc.allow_low_precision(reason="8-term sibling sum"):
                    t4 = [addp.tile([C, PT], bf16, tag="add", name=f"t4_{j}")
                          for j in range(4)]
                    for j in range(4):
                        nc.vector.tensor_tensor(
                            t4[j][:, :], sl(2 * j), sl(2 * j + 1), op=Add)
                    s0 = addp.tile([C, PT], bf16, tag="add", name="s0")
                    nc.vector.tensor_tensor(s0[:, :], t4[0][:, :],
                                            t4[1][:, :], op=Add)
                    s1 = addp.tile([C, PT], bf16, tag="add", name="s1")
                    nc.vector.tensor_tensor(s1[:, :], t4[2][:, :],
                                            t4[3][:, :], op=Add)
                    aggt = aggp.tile([C, PT], bf16, tag="agg", name="aggd")
                    nc.vector.tensor_tensor(aggt[:, :], s0[:, :],
                                            s1[:, :], op=Add)
                nc.tensor.matmul(psum, w["w1a"], aggt[:, :],
                                 start=True, stop=False)
                nc.tensor.matmul(psum, w["w1b"], self_ap,
                                 start=False, stop=True)

            def w3b_tile(t):
                # h2 rows [512t, 512(t+1)) have no in-edges: self term only
                psn = pp.tile([C, PT], f32, tag="ps", name=f"psn{t}")
                nc.tensor.matmul(psn, w["w3b"],
                                 h2sb[:, PT * t: PT * (t + 1)],
                                 start=True, stop=True)
                on = opool.tile([C, PT], f32, tag="o", name=f"on{t}")
                nc.scalar.activation(on[:, :], psn, Relu, bias=bt["b3"])
                nc.sync.dma_start(out[:, PT * t: PT * (t + 1)], on[:, :])

            # layer-0 tile 0 (seeds) early: only needs xA01; fills DVE
            # while the first Ct/D DMAs stream.
            ps0z = pp.tile([C, PT], f32, tag="ps", name="ps0z")
            dve_sage(ps0z, "w1a", "w1b", xA01[:, S:NPAR1], xA01[:, 0:S])
            nc.scalar.activation(h1self[:, 0:S], ps0z, Relu, bias=bt["b1"])

            # Software-pipelined main loop: tile t's DVE-dependent matmuls
            # (layer-1 of t-1, layer-0-inner of t) are emitted AFTER tile
            # t's leaf matmul burst so the PE never waits on a reduce.
            h1tmp_prev = None
            for t in range(1, 9):
                Ct = cpool.tile([C, 8 * PT], dt_in, tag="C")
                nc.sync.dma_start(
                    Ct[:, :], xiT[:, S + 8 * PT * t: S + 8 * PT * (t + 1)])
                Dh = []
                for h in range(2):
                    Dx = dpool.tile([C, LB // 2], f8e3, tag="D",
                                    name=f"D{t}_{h}")
                    nc.sync.dma_start(
                        Dx[:, :], xlT[:, LB * (t - 1) + (LB // 2) * h:
                                      LB * (t - 1) + (LB // 2) * (h + 1)])
                    Dh.append(Dx)

                # DVE queue: L1(t-1) reduce (input ready), L0(t) reduce
                if h1tmp_prev is not None:
                    agg1p = dve_reduce(h1tmp_prev[:, :], "agg1", f"a1_{t}")
                agg0 = dve_reduce(Ct[:, :], "agg0", f"a0_{t}")

                # PE queue: ready-first. w3b(t-2), then the leaf burst.
                if t >= 3:
                    w3b_tile(t - 2)

                h1tmp = hpool.tile([C, 8 * PT], bf16, tag="h1tmp")
                for u in range(8):
                    psu = pp.tile([C, PT], f32, tag="ps", name=f"psu{t}_{u}")
                    if dve_flag[8 * (t - 1) + u]:
                        leaf_dve_sage(psu, Dh[u // 4], u % 4,
                                      Ct[:, PT * u: PT * (u + 1)])
                    else:
                        leaf_pe_sage(psu, Dh[u // 4], u % 4,
                                     Ct[:, PT * u: PT * (u + 1)])
                    nc.scalar.activation(h1tmp[:, PT * u: PT * (u + 1)], psu,
                                         Relu, bias=bt["b1"])

                # layer-0 tile for parents [512t, 512(t+1)) (hop-1 nodes)
                ps0 = pp.tile([C, PT], f32, tag="ps", name=f"ps0_{t}")
                agg_mms(ps0, "w1a", "w1b", agg0,
                        xA01[:, PT * t: PT * (t + 1)])
                nc.scalar.activation(h1self[:, PT * t: PT * (t + 1)], ps0,
                                     Relu, bias=bt["b1"])

                # layer-1 tile for parents [512(t-1), 512t) -> h2
                if h1tmp_prev is not None:
                    ps1 = pp.tile([C, PT], f32, tag="ps", name=f"ps1_{t}")
                    agg_mms(ps1, "w2a", "w2b", agg1p,
                            h1self[:, PT * (t - 1): PT * t])
                    nc.scalar.activation(h2sb[:, PT * (t - 1): PT * t], ps1,
                                         Relu, bias=bt["b2"])
                h1tmp_prev = h1tmp

            # drain: layer-1 tile 8, tile 0, then layer 2
            agg1p = dve_reduce(h1tmp_prev[:, :], "agg1", "a1_9")
            ps1 = pp.tile([C, PT], f32, tag="ps", name="ps1_9")
            agg_mms(ps1, "w2a", "w2b", agg1p, h1self[:, 8 * PT: 9 * PT])
            nc.scalar.activation(h2sb[:, 8 * PT: 9 * PT], ps1,
                                 Relu, bias=bt["b2"])
            w3b_tile(7)

            # layer-1 tile 0: children h1[512:4608) = h1self slice
            ps1z = pp.tile([C, PT], f32, tag="ps", name="ps1z")
            dve_sage(ps1z, "w2a", "w2b", h1self[:, S:NPAR1], h1self[:, 0:S])
            nc.scalar.activation(h2sb[:, 0:S], ps1z, Relu, bias=bt["b2"])
            w3b_tile(8)

            # layer 2: parents [0, 512) aggregate h2[512:4608)
            ps2 = pp.tile([C, PT], f32, tag="ps", name="ps2")
            dve_sage(ps2, "w3a", "w3b", h2sb[:, S:NPAR1], h2sb[:, 0:S])
            o0 = opool.tile([C, PT], f32, tag="o", name="o0")
            nc.scalar.activation(o0[:, :], ps2, Relu, bias=bt["b3"])
            nc.sync.dma_start(out[:, 0:S], o0[:, :])

    nc.compile()
    return nc


def _get_bass(ndve, inner8):
    key = (ndve, inner8)
    if key not in _BASS_CACHE:
        _BASS_CACHE[key] = _build_bass(ndve, inner8)
    return _BASS_CACHE[key]


def _edge_is_tree(edge):
    if edge.shape != (2, E_FULL):
        return False
    ar = np.arange(E_FULL, dtype=np.int64)
    return (np.array_equal(edge[0], (B + ar).astype(np.int32))
            and np.array_equal(edge[1], (ar // 8).astype(np.int32)))


def _fallback(x, edge, W1, b1, W2, b2, W3, b3):
    # General (structure-agnostic) CPU implementation; only used if the
    # inputs are not the fanout-8 tree this kernel is specialized for.
    sizes = [(N_FULL, E_FULL), (299008, 294912), (36864, 32768)]
    params = [(W1, b1), (W2, b2), (W3, b3)]
    x = x.astype(np.float32)
    for (n, e), (Wl, bl) in zip(sizes, params):
        src = edge[0, :e].astype(np.int64)
        dst = edge[1, :e].astype(np.int64)
        x = x[:n]
        agg = np.zeros((n, x.shape[1]), np.float32)
        np.add.at(agg, dst, x[src])
        deg = np.bincount(dst, minlength=n).astype(np.float32)
        agg /= np.maximum(deg, 1.0)[:, None]
        x = np.maximum(np.concatenate([agg, x], axis=1) @ Wl.T + bl, 0.0)
    return x


def kernel(**inputs):
    global LAST_RESULT
    x = np.asarray(inputs["x"])
    edge = np.asarray(inputs["edge"])
    W = [np.asarray(inputs[k], dtype=np.float32) for k in ("W1", "W2", "W3")]
    bias = [np.asarray(inputs[k], dtype=np.float32) for k in ("b1", "b2", "b3")]

    if x.shape != (N_FULL, C) or not _edge_is_tree(edge):
        return _fallback(x, edge, W[0], bias[0], W[1], bias[1], W[2], bias[2])

    import ml_dtypes
    from concourse.bass_utils import run_bass_kernel_spmd

    bf16 = ml_dtypes.bfloat16
    f8e3 = ml_dtypes.float8_e3m4
    x = np.ascontiguousarray(x, dtype=np.float32)

    wblocks = []
    for li in range(3):
        wblocks.append((W[li][:, :C] / 8.0).T)     # agg part, mean folded in
        wblocks.append(W[li][:, C:].T)             # self part
    wconsts = np.concatenate(wblocks, axis=1).astype(bf16)
    wconsts = np.ascontiguousarray(wconsts)
    bconsts = np.ascontiguousarray(np.stack(bias, axis=1))      # [128, 3] f32

    in_maps = []
    for c in range(N_CORES):
        xi = np.concatenate(
            [x[OFF[h] + BLK[h] * c: OFF[h] + BLK[h] * (c + 1)]
             for h in range(3)], axis=0)
        xiTc = np.ascontiguousarray(xi.T).astype(f8e3 if INNER8 else bf16, copy=False)
        xl = x[OFF[3] + BLK[3] * c: OFF[3] + BLK[3] * (c + 1)]
        # per outer tile, split parents into 2 halves; within each half,
        # plane-major: [8 tiles, 2 halves, 8 sib, 2048 parents, 128ch]
        xl = xl.reshape(8, 2, 2048, 8, C).transpose(0, 1, 3, 2, 4)
        xl = xl.reshape(-1, C)
        xlTc = np.ascontiguousarray(xl.T).astype(f8e3, copy=False)
        in_maps.append({"xiT": xiTc, "xlT": xlTc,
                        "wconsts": wconsts, "bconsts": bconsts})

    nc = _get_bass(NDVE, INNER8)
    res = run_bass_kernel_spmd(nc, in_maps, list(range(N_CORES)), trace=TRACE)
    LAST_RESULT = res

    out = np.empty((OUT_ROWS, C), np.float32)
    for c in range(N_CORES):
        oc = np.asarray(res.results[c]["out"])
        out[S * c: S * (c + 1)] = oc[:, :S].T
        out[B + 8 * S * c: B + 8 * S * (c + 1)] = oc[:, S:].T
    return out


# revision 7
# speedup vs baseline: 1.1343x; 1.0212x over previous
"""Trainium2 Bass kernel for 3-layer CuGraphSAGE on a fanout-8 sampled tree.

The sampled graph is a forest of B=4096 independent trees (children of
parent p are rows [4096+8p, 4096+8p+8)). Shard by seed block: core c gets
512 seeds plus their full 3-hop subtrees (contiguous row blocks, exactly
1/8 of all rows, zero halo, no collectives).

Precision: leaf-hop features (87.5% of bytes) stream as fp8 e3m4, inner
hops as bf16, weights bf16, PSUM accumulation f32. Measured end-to-end
rel err ~4e-3 vs the f32 reference.

Layout: channel-major [128ch, rows] so the matmul contraction dim is the
partition dim. Leaf columns are host-reordered PLANE-major per outer tile
(all sibling-0 cols, then sibling-1, ...) so the mean-aggregation is 8
accumulating matmuls with fully CONTIGUOUS 512-col rhs slabs -- strided
rhs runs 2-4x slower on the PE, contiguous hits 216ns/512col. The 1/8 is
folded into the aggregation weight. Inner aggregations (natural order,
sibling-adjacent) use the DVE reduce (4.4us/tile, dtype-independent); a
few leaf sub-tiles are offloaded to DVE via packed plane-adds to balance
engines. Only x is streamed from HBM (~43 MB/core).
"""

import os
import numpy as np

# ---------------------------------------------------------------- constants
N_CORES = 8
C = 128                       # channels
B = 4096                      # seeds
S = B // N_CORES              # 512 seeds per core
BLK = [512, 4096, 32768, 262144]          # per-core rows per hop
OFF = [0, 4096, 36864, 299008]            # global start row of each hop block
NIN = BLK[0] + BLK[1] + BLK[2]            # 37376 inner rows (hop 0-2)
NLEAF = BLK[3]                            # 262144 leaf rows
NPAR1 = BLK[0] + BLK[1]                   # 4608 layer-1 parents
PT = 512                                  # parents per PSUM tile
LB = 8 * BLK[1]                           # 32768 leaf rows per outer tile
N_FULL = 2396160
E_FULL = 2392064
OUT_ROWS = 36864

TRACE = os.environ.get("GNN_TRACE", "0") == "1"
# number of leaf sub-tiles (of 64) aggregated on DVE instead of PE
NDVE = int(os.environ.get("GNN_NDVE", "10"))
INNER8 = os.environ.get("GNN_INNER8", "0") == "1"   # inner hops fp8e3 too
LAST_RESULT = None

_BASS_CACHE = {}


def _leaf_dve_flags(ndve):
    # spread ndve True flags evenly over the 64 leaf sub-tiles
    return [(m * ndve) // 64 != ((m + 1) * ndve) // 64 for m in range(64)]


def _build_bass(ndve, inner8):
    import concourse.mybir as mybir
    from concourse import bacc
    from concourse.tile import TileContext

    bf16 = mybir.dt.bfloat16
    f8e3 = mybir.dt.float8e3
    f32 = mybir.dt.float32
    Relu = mybir.ActivationFunctionType.Relu
    AxX = mybir.AxisListType.X
    Add = mybir.AluOpType.add

    dve_flag = _leaf_dve_flags(ndve)
    dt_in = f8e3 if inner8 else bf16

    nc = bacc.Bacc()
    xiT = nc.dram_tensor("xiT", [C, NIN], dt_in, kind="ExternalInput")
    xlT = nc.dram_tensor("xlT", [C, NLEAF], f8e3, kind="ExternalInput")
    wconsts = nc.dram_tensor("wconsts", [C, 6 * C], bf16, kind="ExternalInput")
    bconsts = nc.dram_tensor("bconsts", [C, 3], f32, kind="ExternalInput")
    out = nc.dram_tensor("out", [C, NPAR1], f32, kind="ExternalOutput")
    WIDX = {k: i for i, k in
            enumerate(("w1a", "w1b", "w2a", "w2b", "w3a", "w3b"))}

    with TileContext(nc) as tc:
        with tc.tile_pool(name="const", bufs=1) as constp, \
             tc.tile_pool(name="keep", bufs=1) as keepp, \
             tc.tile_pool(name="cbuf", bufs=2) as cpool, \
             tc.tile_pool(name="dbuf", bufs=4) as dpool, \
             tc.tile_pool(name="hbuf", bufs=2) as hpool, \
             tc.tile_pool(name="obuf", bufs=2) as opool, \
             tc.tile_pool(name="aggbuf", bufs=3) as aggp, \
             tc.tile_pool(name="addbuf", bufs=8) as addp, \
             tc.tile_pool(name="ps", bufs=8, space="PSUM") as pp:

            wtile = constp.tile([C, 6 * C], bf16, name="wtile")
            nc.sync.dma_start(wtile[:, :], wconsts[:, :])
            btile = constp.tile([C, 3], f32, name="btile")
            nc.sync.dma_start(btile[:, :], bconsts[:, :])
            w = {k: wtile[:, C * i: C * (i + 1)] for k, i in WIDX.items()}
            bt = {f"b{i+1}": btile[:, i: i + 1] for i in range(3)}

            xA01 = keepp.tile([C, NPAR1], dt_in, tag="xA01")
            nc.sync.dma_start(xA01[:, :], xiT[:, 0:NPAR1])
            h1self = keepp.tile([C, NPAR1], bf16, tag="h1self")
            h2sb = keepp.tile([C, NPAR1], bf16, tag="h2sb")
            agg18 = keepp.tile([C, PT], bf16, tag="agg18")   # L1 tile 8
            agg2 = keepp.tile([C, PT], bf16, tag="agg2")     # L2

            def half_reduce(dst_ap, children_ap):
                # 256-parent partial group-reduce into a persistent tile
                with nc.allow_low_precision(reason="8-term sibling sum"):
                    nc.vector.reduce_sum(
                        dst_ap,
                        children_ap.rearrange("c (p e) -> c p e", e=8),
                        axis=AxX)

            def dve_reduce(children_ap, tag, name):
                # DVE group-reduce over sibling-adjacent natural order
                aggt = aggp.tile([C, PT], bf16, tag=tag, name=name)
                with nc.allow_low_precision(reason="8-term sibling sum"):
                    nc.vector.reduce_sum(
                        aggt[:, :],
                        children_ap.rearrange("c (p e) -> c p e", e=8),
                        axis=AxX)
                return aggt


            def agg_mms(psum, wa, wb, aggt, self_ap):
                nc.tensor.matmul(psum, w[wa], aggt[:, :],
                                 start=True, stop=False)
                nc.tensor.matmul(psum, w[wb], self_ap,
                                 start=False, stop=True)

            def dve_sage(psum, wa, wb, children_ap, self_ap):
                aggt = dve_reduce(children_ap, "agg", "aggt")
                agg_mms(psum, wa, wb, aggt, self_ap)

            HW2 = 4 * PT                       # 2048 parents per half

            def leaf_slab(Dx, v, e):
                return Dx[:, HW2 * e + PT * v: HW2 * e + PT * (v + 1)]

            def leaf_pe_sage(psum, Dx, v, self_ap):
                # 8 accumulating mms over contiguous plane slabs
                for e in range(8):
                    nc.tensor.matmul(psum, w["w1a"], leaf_slab(Dx, v, e),
                                     start=(e == 0), stop=False)
                nc.tensor.matmul(psum, w["w1b"], self_ap,
                                 start=False, stop=True)

            def leaf_dve_sage(psum, Dx, v, self_ap):
                # packed plane-adds: 4x (fp8+fp8->bf16), then 2+1 bf16
                def sl(e):
                    return leaf_slab(Dx, v, e)
                with nc.allow_low_precision(reason="8-term sibling sum"):
                    t4 = [addp.tile([C, PT], bf16, tag="add", name=f"t4_{j}")
                          for j in range(4)]
                    for j in range(4):
                        nc.vector.tensor_tensor(
                            t4[j][:, :], sl(2 * j), sl(2 * j + 1), op=Add)
                    s0 = addp.tile([C, PT], bf16, tag="add", name="s0")
                    nc.vector.tensor_tensor(s0[:, :], t4[0][:, :],
                                            t4[1][:, :], op=Add)
                    s1 = addp.tile([C, PT], bf16, tag="add", name="s1")
                    nc.vector.tensor_tensor(s1[:, :], t4[2][:, :],
                                            t4[3][:, :], op=Add)
                    aggt = aggp.tile([C, PT], bf16, tag="agg", name="aggd")
                    nc.vector.tensor_tensor(aggt[:, :], s0[:, :],
                                            s1[:, :], op=Add)
                nc.tensor.matmul(psum, w["w1a"], aggt[:, :],
                                 start=True, stop=False)
                nc.tensor.matmul(psum, w["w1b"], self_ap,
                                 start=False, stop=True)

            def w3b_tile(t):
                # h2 rows [512t, 512(t+1)) have no in-edges: self term only
                psn = pp.tile([C, PT], f32, tag="ps", name=f"psn{t}")
                nc.tensor.matmul(psn, w["w3b"],
                                 h2sb[:, PT * t: PT * (t + 1)],
                                 start=True, stop=True)
                on = opool.tile([C, PT], f32, tag="o", name=f"on{t}")
                nc.scalar.activation(on[:, :], psn, Relu, bias=bt["b3"])
                nc.sync.dma_start(out[:, PT * t: PT * (t + 1)], on[:, :])

            # layer-0 tile 0 (seeds) early: only needs xA01; fills DVE
            # while the first Ct/D DMAs stream.
            ps0z = pp.tile([C, PT], f32, tag="ps", name="ps0z")
            dve_sage(ps0z, "w1a", "w1b", xA01[:, S:NPAR1], xA01[:, 0:S])
            nc.scalar.activation(h1self[:, 0:S], ps0z, Relu, bias=bt["b1"])

            # Software-pipelined main loop: tile t's DVE-dependent matmuls
            # (layer-1 of t-1, layer-0-inner of t) are emitted AFTER tile
            # t's leaf matmul burst so the PE never waits on a reduce.
            h1tmp_prev = None
            for t in range(1, 9):
                Ct = cpool.tile([C, 8 * PT], dt_in, tag="C")
                nc.sync.dma_start(
                    Ct[:, :], xiT[:, S + 8 * PT * t: S + 8 * PT * (t + 1)])
                Dh = []
                for h in range(2):
                    Dx = dpool.tile([C, LB // 2], f8e3, tag="D",
                                    name=f"D{t}_{h}")
                    nc.sync.dma_start(
                        Dx[:, :], xlT[:, LB * (t - 1) + (LB // 2) * h:
                                      LB * (t - 1) + (LB // 2) * (h + 1)])
                    Dh.append(Dx)

                # DVE queue: L1(t-1) reduce (input ready), L0(t) reduce
                if h1tmp_prev is not None:
                    agg1p = dve_reduce(h1tmp_prev[:, :], "agg1", f"a1_{t}")
                agg0 = dve_reduce(Ct[:, :], "agg0", f"a0_{t}")

                # PE queue: ready-first. w3b(t-2), then the leaf burst.
                if t >= 3:
                    w3b_tile(t - 2)

                h1tmp = hpool.tile([C, 8 * PT], bf16, tag="h1tmp")
                for u in range(8):
                    psu = pp.tile([C, PT], f32, tag="ps", name=f"psu{t}_{u}")
                    if dve_flag[8 * (t - 1) + u]:
                        leaf_dve_sage(psu, Dh[u // 4], u % 4,
                                      Ct[:, PT * u: PT * (u + 1)])
                    else:
                        leaf_pe_sage(psu, Dh[u // 4], u % 4,
                                     Ct[:, PT * u: PT * (u + 1)])
                    nc.scalar.activation(h1tmp[:, PT * u: PT * (u + 1)], psu,
                                         Relu, bias=bt["b1"])
                    if t == 8 and u in (3, 7):
                        hh = (u - 3) // 4
                        half_reduce(agg18[:, 256 * hh: 256 * (hh + 1)],
                                    h1tmp[:, 4 * PT * hh: 4 * PT * (hh + 1)])

                # layer-0 tile for parents [512t, 512(t+1)) (hop-1 nodes)
                ps0 = pp.tile([C, PT], f32, tag="ps", name=f"ps0_{t}")
                agg_mms(ps0, "w1a", "w1b", agg0,
                        xA01[:, PT * t: PT * (t + 1)])
                nc.scalar.activation(h1self[:, PT * t: PT * (t + 1)], ps0,
                                     Relu, bias=bt["b1"])
                if t == 8:
                    # L1 tile 0 aggregation: h1self[512:4608) now complete
                    agg1z = dve_reduce(h1self[:, S:NPAR1], "agg1z", "a1z")

                # layer-1 tile for parents [512(t-1), 512t) -> h2
                if h1tmp_prev is not None:
                    ps1 = pp.tile([C, PT], f32, tag="ps", name=f"ps1_{t}")
                    agg_mms(ps1, "w2a", "w2b", agg1p,
                            h1self[:, PT * (t - 1): PT * t])
                    nc.scalar.activation(h2sb[:, PT * (t - 1): PT * t], ps1,
                                         Relu, bias=bt["b2"])
                    if t == 5:
                        # first half of L2 agg: h2 slices 1-4 complete
                        half_reduce(agg2[:, 0:256], h2sb[:, PT: 5 * PT])
                h1tmp_prev = h1tmp

            # drain: agg18/agg1z were built during tile 8
            ps1 = pp.tile([C, PT], f32, tag="ps", name="ps1_9")
            agg_mms(ps1, "w2a", "w2b", agg18, h1self[:, 8 * PT: 9 * PT])
            nc.scalar.activation(h2sb[:, 8 * PT: 9 * PT], ps1,
                                 Relu, bias=bt["b2"])
            w3b_tile(7)

            # layer-1 tile 0
            ps1z = pp.tile([C, PT], f32, tag="ps", name="ps1z")
            agg_mms(ps1z, "w2a", "w2b", agg1z, h1self[:, 0:S])
            nc.scalar.activation(h2sb[:, 0:S], ps1z, Relu, bias=bt["b2"])
            w3b_tile(8)

            # layer 2: second half of agg (h2 slices 5-8), then the mms
            half_reduce(agg2[:, 256:512], h2sb[:, 5 * PT: 9 * PT])
            ps2 = pp.tile([C, PT], f32, tag="ps", name="ps2")
            agg_mms(ps2, "w3a", "w3b", agg2, h2sb[:, 0:S])
            o0 = opool.tile([C, PT], f32, tag="o", name="o0")
            nc.scalar.activation(o0[:, :], ps2, Relu, bias=bt["b3"])
            nc.sync.dma_start(out[:, 0:S], o0[:, :])

    nc.compile()
    return nc


def _get_bass(ndve, inner8):
    key = (ndve, inner8)
    if key not in _BASS_CACHE:
        _BASS_CACHE[key] = _build_bass(ndve, inner8)
    return _BASS_CACHE[key]


def _edge_is_tree(edge):
    if edge.shape != (2, E_FULL):
        return False
    ar = np.arange(E_FULL, dtype=np.int64)
    return (np.array_equal(edge[0], (B + ar).astype(np.int32))
            and np.array_equal(edge[1], (ar // 8).astype(np.int32)))


def _fallback(x, edge, W1, b1, W2, b2, W3, b3):
    # General (structure-agnostic) CPU implementation; only used if the
    # inputs are not the fanout-8 tree this kernel is specialized for.
    sizes = [(N_FULL, E_FULL), (299008, 294912), (36864, 32768)]
    params = [(W1, b1), (W2, b2), (W3, b3)]
    x = x.astype(np.float32)
    for (n, e), (Wl, bl) in zip(sizes, params):
        src = edge[0, :e].astype(np.int64)
        dst = edge[1, :e].astype(np.int64)
        x = x[:n]
        agg = np.zeros((n, x.shape[1]), np.float32)
        np.add.at(agg, dst, x[src])
        deg = np.bincount(dst, minlength=n).astype(np.float32)
        agg /= np.maximum(deg, 1.0)[:, None]
        x = np.maximum(np.concatenate([agg, x], axis=1) @ Wl.T + bl, 0.0)
    return x


def kernel(**inputs):
    global LAST_RESULT
    x = np.asarray(inputs["x"])
    edge = np.asarray(inputs["edge"])
    W = [np.asarray(inputs[k], dtype=np.float32) for k in ("W1", "W2", "W3")]
    bias = [np.asarray(inputs[k], dtype=np.float32) for k in ("b1", "b2", "b3")]

    if x.shape != (N_FULL, C) or not _edge_is_tree(edge):
        return _fallback(x, edge, W[0], bias[0], W[1], bias[1], W[2], bias[2])

    import ml_dtypes
    from concourse.bass_utils import run_bass_kernel_spmd

    bf16 = ml_dtypes.bfloat16
    f8e3 = ml_dtypes.float8_e3m4
    x = np.ascontiguousarray(x, dtype=np.float32)

    wblocks = []
    for li in range(3):
        wblocks.append((W[li][:, :C] / 8.0).T)     # agg part, mean folded in
        wblocks.append(W[li][:, C:].T)             # self part
    wconsts = np.concatenate(wblocks, axis=1).astype(bf16)
    wconsts = np.ascontiguousarray(wconsts)
    bconsts = np.ascontiguousarray(np.stack(bias, axis=1))      # [128, 3] f32

    in_maps = []
    for c in range(N_CORES):
        xi = np.concatenate(
            [x[OFF[h] + BLK[h] * c: OFF[h] + BLK[h] * (c + 1)]
             for h in range(3)], axis=0)
        xiTc = np.ascontiguousarray(xi.T).astype(f8e3 if INNER8 else bf16, copy=False)
        xl = x[OFF[3] + BLK[3] * c: OFF[3] + BLK[3] * (c + 1)]
        # per outer tile, split parents into 2 halves; within each half,
        # plane-major: [8 tiles, 2 halves, 8 sib, 2048 parents, 128ch]
        xl = xl.reshape(8, 2, 2048, 8, C).transpose(0, 1, 3, 2, 4)
        xl = xl.reshape(-1, C)
        xlTc = np.ascontiguousarray(xl.T).astype(f8e3, copy=False)
        in_maps.append({"xiT": xiTc, "xlT": xlTc,
                        "wconsts": wconsts, "bconsts": bconsts})

    nc = _get_bass(NDVE, INNER8)
    res = run_bass_kernel_spmd(nc, in_maps, list(range(N_CORES)), trace=TRACE)
    LAST_RESULT = res

    out = np.empty((OUT_ROWS, C), np.float32)
    for c in range(N_CORES):
        oc = np.asarray(res.results[c]["out"])
        out[S * c: S * (c + 1)] = oc[:, :S].T
        out[B + 8 * S * c: B + 8 * S * (c + 1)] = oc[:, S:].T
    return out
